# revision 4
# baseline (speedup 1.0000x reference)
"""Multi-head attention (B=4, P=2048, D=1024, H=16) on 8 TRN2 NeuronCores.

Sharding: tensor-parallel over heads (2 heads per core). Each core computes
qkv for its heads, full attention for its heads, and a partial output
projection (rows of w_proj for its heads). Partials are summed on host.

Per-core dataflow (all row counts R = B*P = 8192):
  - xT (D, R) f32r in DRAM, streamed in column chunks of 512.
  - qkv^T = W_shard^T @ x^T via PE (stationary = W block, moving = xT chunk).
    q^T, k^T stored [128 part = 2 heads x 64, rows] per batch; v^T transposed
    on PE to v natural [row, d] bf16 with an appended ones column.
  - scores^T = K @ q^T per (head, batch): K=64 matmuls (PE row-tiling puts
    head0 on partitions 0-63 / T0, head1 on 64-127 / T8).
  - E = exp(scale * scores^T) on ACT -> bf16.
  - o = E^T-stationary matmul with [v | 1] moving: gives o and softmax
    denominators in one accumulation group. Normalize via ACT copy with
    per-partition reciprocal scale.
  - both heads' normalized o packed [128 q, 128 hd] -> one PE transpose
    -> o^T [128 hd, q] with head partition placement matching w_proj rows.
  - partial = o^T.T @ w_proj_shard -> DMA out.
"""

import numpy as np

import concourse.bass as bass
import concourse.tile as tile
from concourse import bacc, mybir
from concourse import bass_utils
from concourse.masks import make_identity

B, P, D = 4, 2048, 1024
H = 16
NCORES = 8
HPC = H // NCORES          # heads per core = 2
d = D // H                 # 64
R = B * P                  # 8192
SCALE = float(d) ** -0.5

F32 = mybir.dt.float32
F32R = mybir.dt.float32r
BF16 = mybir.dt.bfloat16
AF = mybir.ActivationFunctionType

_CACHE = {}


def _build():
    nc = bacc.Bacc("TRN2", target_bir_lowering=False, debug=False,
                   enable_asserts=False)
    xT = nc.dram_tensor("xT", (D, R), F32R, kind="ExternalInput").ap()
    wqkv = nc.dram_tensor("wqkv", (128, 3072), F32R, kind="ExternalInput").ap()
    wproj = nc.dram_tensor("wproj", (128, D), F32R, kind="ExternalInput").ap()
    out = nc.dram_tensor("out", (R, D), F32, kind="ExternalOutput").ap()

    # partition-first on the SBUF side; DRAM side reordered to match the
    # element iteration order (p, kb, n) / (p, r, n)
    xT3 = xT.rearrange("(kb p) n -> p kb n", p=128)      # [128, 8, 8192]
    out3 = out.rearrange("(r p) n -> p r n", p=128)      # [128, 64, 1024]

    with tile.TileContext(nc) as tc:
        from contextlib import ExitStack
        with ExitStack() as ctx:
            p_const = ctx.enter_context(tc.tile_pool(name="const", bufs=1))
            p_w = ctx.enter_context(tc.tile_pool(name="w", bufs=1))
            p_x = ctx.enter_context(tc.tile_pool(name="x", bufs=2))
            p_qk = ctx.enter_context(tc.tile_pool(name="qk", bufs=2))
            p_v = ctx.enter_context(tc.tile_pool(name="v", bufs=2))
            p_vt = ctx.enter_context(tc.tile_pool(name="vt", bufs=2))
            p_e = ctx.enter_context(tc.tile_pool(name="e", bufs=48))
            p_ot = ctx.enter_context(tc.tile_pool(name="ot", bufs=2))
            p_on = ctx.enter_context(tc.tile_pool(name="on", bufs=2))
            p_r = ctx.enter_context(tc.tile_pool(name="r", bufs=4))
            p_out = ctx.enter_context(tc.tile_pool(name="o", bufs=2))
            ps_big = ctx.enter_context(
                tc.tile_pool(name="psb", bufs=2, space="PSUM"))
            ps_t = ctx.enter_context(
                tc.tile_pool(name="pst", bufs=2, space="PSUM"))
            ps_s = ctx.enter_context(
                tc.tile_pool(name="pss", bufs=2, space="PSUM"))
            ps_o = ctx.enter_context(
                tc.tile_pool(name="pso", bufs=2, space="PSUM"))

            ident = p_const.tile([128, 128], F32)
            make_identity(nc, ident[:])

            wq_sb = p_w.tile([128, 3072], F32R)
            nc.sync.dma_start(wq_sb[:], wqkv[:])
            wp_sb = p_w.tile([128, D], F32R)
            nc.sync.dma_start(wp_sb[:], wproj[:])

            for b in range(B):
                qt = p_qk.tile([128, P], F32R, tag="qt")
                kt = p_qk.tile([128, P], F32R, tag="kt")
                vt_b = p_v.tile([128, 2 * 16 * 65], BF16, tag="v")
                ones_view = vt_b.rearrange("p (blk w) -> p blk w", w=65)
                nc.vector.memset(ones_view[:, :, 64:65], 1.0)

                # ---- stage A: qkv for this batch's 4 row-chunks of 512 ----
                for cc in range(4):
                    c = b * 4 + cc
                    xt = p_x.tile([128, 8 * 512], F32R)
                    nc.sync.dma_start(
                        xt.rearrange("p (kb n) -> p kb n", n=512),
                        xT3[:, :, c * 512:(c + 1) * 512])
                    for m in range(3):
                        ps = ps_big.tile([128, 512], F32, tag="big")
                        for kb in range(8):
                            col = kb * 384 + m * 128
                            nc.tensor.matmul(
                                ps[:], wq_sb[:, col:col + 128],
                                xt[:, kb * 512:(kb + 1) * 512],
                                start=(kb == 0), stop=(kb == 7))
                        if m == 0:
                            nc.scalar.copy(qt[:, cc * 512:(cc + 1) * 512], ps[:])
                        elif m == 1:
                            nc.vector.tensor_copy(
                                kt[:, cc * 512:(cc + 1) * 512], ps[:])
                        else:
                            vtmp = p_vt.tile([128, 512], F32)
                            nc.vector.tensor_copy(vtmp[:], ps[:])
                            for h in range(2):
                                for rs in range(4):
                                    jb = cc * 4 + rs
                                    pt = ps_t.tile([128, 128], F32, tag="t")
                                    nc.tensor.transpose(
                                        pt[:, 0:64],
                                        vtmp[h * 64:(h + 1) * 64,
                                             rs * 128:(rs + 1) * 128],
                                        ident[h * 64:(h + 1) * 64,
                                              h * 64:(h + 1) * 64])
                                    vcol = (h * 16 + jb) * 65
                                    nc.vector.tensor_copy(
                                        vt_b[:, vcol:vcol + 64], pt[:, 0:64])

                # ---- attention for this batch ----
                ot_b = p_ot.tile([128, P], F32R, tag="ot")
                for ic in range(4):
                    e_tiles = [[None] * 16 for _ in range(2)]
                    for jb in range(16):
                        for h in range(2):
                            pss = ps_s.tile([128, 512], F32, tag="s")
                            nc.tensor.matmul(
                                pss[:],
                                kt[h * 64:(h + 1) * 64,
                                   jb * 128:(jb + 1) * 128],
                                qt[h * 64:(h + 1) * 64,
                                   ic * 512:(ic + 1) * 512],
                                start=True, stop=True)
                            et = p_e.tile([128, 512], BF16, tag="e")
                            nc.scalar.activation(et[:], pss[:], AF.Exp,
                                                 scale=SCALE)
                            e_tiles[h][jb] = et
                    for ib in range(4):
                        psos, recips = [], []
                        for h in range(2):
                            pso = ps_o.tile([128, 65], F32, tag="o")
                            for jb in range(16):
                                vcol = (h * 16 + jb) * 65
                                nc.tensor.matmul(
                                    pso[:],
                                    e_tiles[h][jb][:, ib * 128:(ib + 1) * 128],
                                    vt_b[:, vcol:vcol + 65],
                                    start=(jb == 0), stop=(jb == 15))
                            rcp = p_r.tile([128, 1], F32, tag="r")
                            nc.vector.reciprocal(rcp[:], pso[:, 64:65])
                            psos.append(pso)
                            recips.append(rcp)
                        o_n = p_on.tile([128, 128], F32, tag="on")
                        for h in range(2):
                            nc.scalar.activation(
                                o_n[:, h * 64:(h + 1) * 64],
                                psos[h][:, 0:64], AF.Copy, scale=recips[h][:])
                        pt2 = ps_t.tile([128, 128], F32, tag="t")
                        nc.tensor.transpose(pt2[:], o_n[:], ident[:])
                        nc.vector.tensor_copy(
                            ot_b[:, ic * 512 + ib * 128:
                                 ic * 512 + (ib + 1) * 128], pt2[:])

                # ---- partial output projection for this batch ----
                for rr in range(8):
                    outsb = p_out.tile([128, 2048], F32, tag="os")
                    for half in range(2):
                        rb = rr * 2 + half
                        for n2 in range(2):
                            psp = ps_big.tile([128, 512], F32, tag="big")
                            nc.tensor.matmul(
                                psp[:], ot_b[:, rb * 128:(rb + 1) * 128],
                                wp_sb[:, n2 * 512:(n2 + 1) * 512],
                                start=True, stop=True)
                            nc.vector.tensor_copy(
                                outsb[:, half * 1024 + n2 * 512:
                                      half * 1024 + (n2 + 1) * 512], psp[:])
                    r0 = b * 16 + rr * 2
                    nc.sync.dma_start(
                        out3[:, r0:r0 + 2, :],
                        outsb.rearrange("p (r n) -> p r n", n=1024))

    nc.compile()
    return nc


def _in_maps(x, w_qkv, w_proj):
    x2 = np.ascontiguousarray(x.reshape(R, D).T).astype(np.float32)  # (D, R)
    Wq = w_qkv.reshape(D, 3, H, d)
    Wp = w_proj.reshape(H, d, D)
    maps = []
    for c in range(NCORES):
        hs = slice(c * HPC, (c + 1) * HPC)
        # per-core qkv weight shard, columns ordered (qkv, head, d)
        w_shard = np.ascontiguousarray(Wq[:, :, hs, :]).reshape(D, 3 * HPC * d)
        # pre-tile: [p, kb*384 + m*128 + col] = w_shard[kb*128+p, m*128+col]
        wq_pre = np.ascontiguousarray(
            w_shard.reshape(8, 128, 3, 128).transpose(1, 0, 2, 3)
        ).reshape(128, 3072)
        wp_shard = np.ascontiguousarray(Wp[hs]).reshape(HPC * d, D)
        maps.append({
            "xT": x2,
            "wqkv": np.ascontiguousarray(wq_pre).astype(np.float32),
            "wproj": wp_shard.astype(np.float32),
        })
    return maps


def get_nc():
    if "nc" not in _CACHE:
        _CACHE["nc"] = _build()
    return _CACHE["nc"]


def kernel(x, w_qkv, w_proj, b_proj):
    x = np.asarray(x)
    w_qkv = np.asarray(w_qkv)
    w_proj = np.asarray(w_proj)
    b_proj = np.asarray(b_proj)
    nc = get_nc()
    maps = _in_maps(x, w_qkv, w_proj)
    res = bass_utils.run_bass_kernel_spmd(nc, maps, core_ids=list(range(NCORES)))
    acc = np.zeros((R, D), dtype=np.float64)
    for r in res.results:
        acc += r["out"].astype(np.float64)
    acc += b_proj.astype(np.float64)
    return acc.reshape(B, P, D).astype(np.float32)
